# revision 4
# baseline (speedup 1.0000x reference)
"""Trainium2 kernel for nn_CompressedLinearRANS: out = x @ (w_int8*scale).T + bias.

Strategy: data-parallel over the 8192 rows of x (1024 rows per core, full
weight replicated per core). Per-core matmul M=1024, K=4096, N=4096 done in
fp16 on the PE array: int8-range weights are exactly representable in fp16,
x is rounded fp32->fp16 on device (the original module computes in fp16).
Scale+bias are applied in fp32 during PSUM eviction.

Inputs are pre-transposed on the host so no on-device transposes are needed:
  xT  [4096, 1024] fp32  per core (shard of x, transposed)
  wT  [4096, 4096] int32 (weight_int8.T, replicated)
  bias_rep [128, 4096] fp32 (bias broadcast to 128 partitions)
  scale_rep [128, 1] fp32
Output per core: out [1024, 4096] fp32.
"""

import os

import numpy as np

import concourse.bass as bass
import concourse.mybir as mybir
import concourse.tile as tile
from concourse.bass_utils import run_bass_kernel_spmd

# ---------------------------------------------------------------------------
# Workaround for this walrus build: it rejects instructions carrying more
# than one sync wait command ("Too many sync wait commands",
# CoreV*GenImpl::setupSyncWait). After Tile lowering, split any instruction
# with >1 sem wait: keep the last wait on the instruction and hoist the rest
# onto same-engine NoOps inserted immediately before it (engines execute
# their stream in order, so this is semantics-preserving).
# ---------------------------------------------------------------------------


def _split_sync_waits(nc, max_waits=1):
    for fn in nc.m.functions:
        for bb in fn.blocks:
            out = []
            changed = False
            for inst in bb.instructions:
                si = inst.sync_info
                waits = list(si.on_wait) if si is not None and si.on_wait else []
                if len(waits) > max_waits:
                    changed = True
                    for w in waits[:-max_waits]:
                        nop = mybir.InstNoOp(
                            name=nc.get_next_instruction_name(),
                            sync_info=mybir.SyncInfo(on_wait=[w], on_update=[]),
                            bass_nofuse=True,
                            engine=inst.engine,
                        )
                        nc.register_instruction(nop)
                        out.append(nop)
                    inst.sync_info = mybir.SyncInfo(
                        on_wait=waits[-max_waits:],
                        on_update=list(si.on_update or []),
                    )
                out.append(inst)
            if changed:
                bb.instructions = out


# ---------------------------------------------------------------------------

P = 128
N_CORES = 8

OUT_F, IN_F = 4096, 4096
B, S = 4, 2048
M_TOTAL = B * S           # 8192 rows
M = M_TOTAL // N_CORES    # 1024 rows per core

F32 = mybir.dt.float32
F16 = mybir.dt.float16
I32 = mybir.dt.int32

# exposed for test.py
LAST_RESULTS = None


def build_nc(M=M, K=IN_F, N=OUT_F, NF=512, trn_type=None):
    """Build the per-core Bass module."""
    KO = K // P    # k chunks of 128
    MT = M // P    # m tiles of 128
    NT = N // NF   # n tiles of NF

    nc = bass.Bass() if trn_type is None else bass.Bass(trn_type)
    xT = nc.dram_tensor("xT", [K, M], F32, kind="ExternalInput")
    wT = nc.dram_tensor("wT", [K, N], I32, kind="ExternalInput")
    bias_rep = nc.dram_tensor("bias_rep", [P, N], F32, kind="ExternalInput")
    scale_rep = nc.dram_tensor("scale_rep", [P, 1], F32, kind="ExternalInput")
    out = nc.dram_tensor("out", [M, N], F32, kind="ExternalOutput")

    x_view = xT.rearrange("(ko p) m -> p ko m", p=P)     # [P, KO, M]
    w_view = wT.rearrange("(ko p) n -> p ko n", p=P)     # [P, KO, N]
    out_view = out.rearrange("(mt p) n -> p mt n", p=P)  # [P, MT, N]

    with tile.TileContext(nc) as tc:
        with (
            tc.tile_pool(name="const", bufs=1) as const_pool,
            tc.tile_pool(name="x16", bufs=1) as x16_pool,
            tc.tile_pool(name="xstage", bufs=3) as xstage_pool,
            tc.tile_pool(name="w16", bufs=2) as w16_pool,
            tc.tile_pool(name="wstage", bufs=4) as wstage_pool,
            tc.tile_pool(name="outsb", bufs=4) as out_pool,
            tc.tile_pool(name="psum", bufs=4, space="PSUM") as psum_pool,
        ):
            bias_sb = const_pool.tile([P, N], F32)
            nc.sync.dma_start(bias_sb[:], bias_rep[:])
            scale_sb = const_pool.tile([P, 1], F32)
            nc.sync.dma_start(scale_sb[:], scale_rep[:])

            # x: load fp32, cast to fp16 resident tile
            x16 = x16_pool.tile([P, KO, M], F16)
            for ko in range(KO):
                xs = xstage_pool.tile([P, M], F32)
                nc.sync.dma_start(xs[:], x_view[:, ko, :])
                nc.scalar.copy(x16[:, ko, :], xs[:])

            for nt in range(NT):
                # weight n-slice: load int32, cast to fp16 (exact for +-127)
                w16 = w16_pool.tile([P, KO, NF], F16)
                for ko in range(KO):
                    ws = wstage_pool.tile([P, NF], I32)
                    nc.sync.dma_start(
                        ws[:], w_view[:, ko, nt * NF : (nt + 1) * NF]
                    )
                    nc.vector.tensor_copy(w16[:, ko, :], ws[:])

                for mt in range(MT):
                    ps = psum_pool.tile([P, NF], F32)
                    for ko in range(KO):
                        nc.tensor.matmul(
                            ps[:],
                            x16[:, ko, mt * P : (mt + 1) * P],
                            w16[:, ko, :],
                            start=(ko == 0),
                            stop=(ko == KO - 1),
                        )
                    ot = out_pool.tile([P, NF], F32)
                    # ot = psum * scale
                    nc.scalar.activation(
                        ot[:], ps[:], mybir.ActivationFunctionType.Copy,
                        scale=scale_sb[:],
                    )
                    # ot += bias
                    nc.vector.tensor_add(
                        ot[:], ot[:], bias_sb[:, nt * NF : (nt + 1) * NF]
                    )
                    nc.sync.dma_start(
                        out_view[:, mt, nt * NF : (nt + 1) * NF], ot[:]
                    )

    _split_sync_waits(nc)
    return nc


def kernel(x, weight_int8, scale, bias):
    global LAST_RESULTS
    x = np.asarray(x)
    weight_int8 = np.asarray(weight_int8)
    scale = np.asarray(scale)
    bias = np.asarray(bias)

    x2d = np.ascontiguousarray(x.reshape(M_TOTAL, IN_F).astype(np.float32, copy=False))
    wT = np.ascontiguousarray(weight_int8.T.astype(np.int32, copy=False))
    bias_rep = np.ascontiguousarray(
        np.broadcast_to(bias.astype(np.float32, copy=False), (P, OUT_F))
    )
    scale_rep = np.full((P, 1), np.float32(scale), dtype=np.float32)

    in_maps = []
    for c in range(N_CORES):
        shard = x2d[c * M : (c + 1) * M]
        in_maps.append(
            {
                "xT": np.ascontiguousarray(shard.T),
                "wT": wT,
                "bias_rep": bias_rep,
                "scale_rep": scale_rep,
            }
        )

    nc = build_nc()
    trace = bool(int(os.environ.get("KERNEL_TRACE", "0")))
    res = run_bass_kernel_spmd(nc, in_maps, list(range(N_CORES)), trace=trace)
    LAST_RESULTS = res

    out = np.empty((M_TOTAL, OUT_F), dtype=np.float32)
    for c in range(N_CORES):
        out[c * M : (c + 1) * M] = res.results[c]["out"]
    return out.reshape(B, S, OUT_F)


# revision 11
# speedup vs baseline: 1.0438x; 1.0438x over previous
"""Trainium2 kernel for nn_CompressedLinearRANS: out = x @ (w_int8*scale).T + bias.

Strategy: data-parallel over the 8192 rows of x (1024 rows per core, full
weight replicated per core). Per-core matmul M=1024, K=4096, N=4096 done in
fp16 on the PE array: int8-range weights are exactly representable in fp16,
x is rounded fp32->fp16 on device (the original module computes in fp16).
Scale+bias are applied in fp32 during PSUM eviction.

Inputs are pre-transposed on the host so no on-device transposes are needed:
  xT  [4096, 1024] fp32  per core (shard of x, transposed)
  wT  [4096, 4096] int32 (weight_int8.T, replicated)
  bias_rep [128, 4096] fp32 (bias broadcast to 128 partitions)
  scale_rep [128, 1] fp32
Output per core: out [1024, 4096] fp32.
"""

import os

import numpy as np

import concourse.bass as bass
import concourse.mybir as mybir
import concourse.tile as tile
from concourse.bass_utils import run_bass_kernel_spmd

# ---------------------------------------------------------------------------
# Workaround for this walrus build: it rejects instructions carrying more
# than one sync wait command ("Too many sync wait commands",
# CoreV*GenImpl::setupSyncWait). After Tile lowering, split any instruction
# with >1 sem wait: keep the last wait on the instruction and hoist the rest
# onto same-engine NoOps inserted immediately before it (engines execute
# their stream in order, so this is semantics-preserving).
# ---------------------------------------------------------------------------


def _split_sync_waits(nc, max_waits=1):
    for fn in nc.m.functions:
        for bb in fn.blocks:
            out = []
            changed = False
            for inst in bb.instructions:
                si = inst.sync_info
                waits = list(si.on_wait) if si is not None and si.on_wait else []
                if len(waits) > max_waits:
                    changed = True
                    for w in waits[:-max_waits]:
                        nop = mybir.InstNoOp(
                            name=nc.get_next_instruction_name(),
                            sync_info=mybir.SyncInfo(on_wait=[w], on_update=[]),
                            bass_nofuse=True,
                            engine=inst.engine,
                        )
                        nc.register_instruction(nop)
                        out.append(nop)
                    inst.sync_info = mybir.SyncInfo(
                        on_wait=waits[-max_waits:],
                        on_update=list(si.on_update or []),
                    )
                out.append(inst)
            if changed:
                bb.instructions = out


# ---------------------------------------------------------------------------

P = 128
N_CORES = 8

OUT_F, IN_F = 4096, 4096
B, S = 4, 2048
M_TOTAL = B * S           # 8192 rows
M = M_TOTAL // N_CORES    # 1024 rows per core

F32 = mybir.dt.float32
F16 = mybir.dt.float16
I8 = mybir.dt.int8

# exposed for test.py
LAST_RESULTS = None


def build_nc(M=M, K=IN_F, N=OUT_F, NF=512, trn_type=None):
    """Build the per-core Bass module."""
    KO = K // P    # k chunks of 128
    MT = M // P    # m tiles of 128
    NT = N // NF   # n tiles of NF

    nc = bass.Bass() if trn_type is None else bass.Bass(trn_type)
    xT = nc.dram_tensor("xT", [K, M], F32, kind="ExternalInput")
    wT = nc.dram_tensor("wT", [K, N], I8, kind="ExternalInput")
    bias_rep = nc.dram_tensor("bias_rep", [P, N], F32, kind="ExternalInput")
    scale_rep = nc.dram_tensor("scale_rep", [P, 1], F32, kind="ExternalInput")
    out = nc.dram_tensor("out", [M, N], F32, kind="ExternalOutput")

    x_view = xT.rearrange("(ko p) m -> p ko m", p=P)     # [P, KO, M]
    w_view = wT.rearrange("(ko p) n -> p ko n", p=P)     # [P, KO, N]
    out_view = out.rearrange("(mt p) n -> p mt n", p=P)  # [P, MT, N]

    with tile.TileContext(nc) as tc:
        with (
            tc.tile_pool(name="const", bufs=1) as const_pool,
            tc.tile_pool(name="x16", bufs=1) as x16_pool,
            tc.tile_pool(name="xstage", bufs=3) as xstage_pool,
            tc.tile_pool(name="w16", bufs=2) as w16_pool,
            tc.tile_pool(name="wstage", bufs=4) as wstage_pool,
            tc.tile_pool(name="outsb", bufs=4) as out_pool,
            tc.tile_pool(name="psum", bufs=8, space="PSUM") as psum_pool,
        ):
            bias_sb = const_pool.tile([P, N], F32)
            nc.sync.dma_start(bias_sb[:], bias_rep[:])
            scale_sb = const_pool.tile([P, 1], F32)
            nc.sync.dma_start(scale_sb[:], scale_rep[:])

            # x: load fp32, cast to fp16 resident tile
            x16 = x16_pool.tile([P, KO, M], F16)
            for ko in range(KO):
                xs = xstage_pool.tile([P, M], F32)
                nc.sync.dma_start(xs[:], x_view[:, ko, :])
                nc.scalar.copy(x16[:, ko, :], xs[:])

            def evict(ps, mt, nt):
                ot = out_pool.tile([P, NF], F32)
                # ot = psum * scale
                nc.scalar.activation(
                    ot[:], ps[:], mybir.ActivationFunctionType.Copy,
                    scale=scale_sb[:],
                )
                # ot += bias
                nc.vector.tensor_add(
                    ot[:], ot[:], bias_sb[:, nt * NF : (nt + 1) * NF]
                )
                nc.sync.dma_start(
                    out_view[:, mt, nt * NF : (nt + 1) * NF], ot[:]
                )

            for nt in range(NT):
                # weight n-slice: load int8, cast to fp16 (exact for +-127)
                w16 = w16_pool.tile([P, KO, NF], F16)
                if nt == 0:
                    # Startup is DMA-paced: go k-outer / m-inner across all
                    # 8 PSUM banks so each arriving (x,w) k-chunk unlocks
                    # MT matmuls instead of one.
                    pss = [
                        psum_pool.tile([P, NF], F32, tag="ps", name=f"ps0_{mt}")
                        for mt in range(MT)
                    ]
                    for ko in range(KO):
                        ws = wstage_pool.tile([P, NF], I8)
                        nc.sync.dma_start(ws[:], w_view[:, ko, 0:NF])
                        nc.vector.tensor_copy(w16[:, ko, :], ws[:])
                        for mt in range(MT):
                            nc.tensor.matmul(
                                pss[mt][:],
                                x16[:, ko, mt * P : (mt + 1) * P],
                                w16[:, ko, :],
                                start=(ko == 0),
                                stop=(ko == KO - 1),
                            )
                    for mt in range(MT):
                        evict(pss[mt], mt, 0)
                else:
                    for ko in range(KO):
                        ws = wstage_pool.tile([P, NF], I8)
                        nc.sync.dma_start(
                            ws[:], w_view[:, ko, nt * NF : (nt + 1) * NF]
                        )
                        nc.vector.tensor_copy(w16[:, ko, :], ws[:])

                    for mt in range(MT):
                        ps = psum_pool.tile([P, NF], F32, tag="ps", name=f"ps{nt}_{mt}")
                        for ko in range(KO):
                            nc.tensor.matmul(
                                ps[:],
                                x16[:, ko, mt * P : (mt + 1) * P],
                                w16[:, ko, :],
                                start=(ko == 0),
                                stop=(ko == KO - 1),
                            )
                        evict(ps, mt, nt)

    _split_sync_waits(nc)
    return nc


def kernel(x, weight_int8, scale, bias):
    global LAST_RESULTS
    x = np.asarray(x)
    weight_int8 = np.asarray(weight_int8)
    scale = np.asarray(scale)
    bias = np.asarray(bias)

    x2d = np.ascontiguousarray(x.reshape(M_TOTAL, IN_F).astype(np.float32, copy=False))
    # int8 range [-127,127]: the int8 pack is exact (this is the module's
    # decoded-int8 weight; int32 is just the container dtype)
    wT = np.ascontiguousarray(weight_int8.T.astype(np.int8))
    bias_rep = np.ascontiguousarray(
        np.broadcast_to(bias.astype(np.float32, copy=False), (P, OUT_F))
    )
    scale_rep = np.full((P, 1), np.float32(scale), dtype=np.float32)

    in_maps = []
    for c in range(N_CORES):
        shard = x2d[c * M : (c + 1) * M]
        in_maps.append(
            {
                "xT": np.ascontiguousarray(shard.T),
                "wT": wT,
                "bias_rep": bias_rep,
                "scale_rep": scale_rep,
            }
        )

    nc = build_nc()
    trace = bool(int(os.environ.get("KERNEL_TRACE", "0")))
    res = run_bass_kernel_spmd(nc, in_maps, list(range(N_CORES)), trace=trace)
    LAST_RESULTS = res

    out = np.empty((M_TOTAL, OUT_F), dtype=np.float32)
    for c in range(N_CORES):
        out[c * M : (c + 1) * M] = res.results[c]["out"]
    return out.reshape(B, S, OUT_F)


# revision 13
# speedup vs baseline: 1.0635x; 1.0189x over previous
"""Trainium2 kernel for nn_CompressedLinearRANS: out = x @ (w_int8*scale).T + bias.

Strategy: data-parallel over the 8192 rows of x (1024 rows per core, full
weight replicated per core). Per-core matmul M=1024, K=4096, N=4096 done in
fp16 on the PE array: int8-range weights are exactly representable in fp16,
x is rounded fp32->fp16 on device (the original module computes in fp16).
Scale+bias are applied in fp32 during PSUM eviction.

Inputs are pre-transposed on the host so no on-device transposes are needed:
  xT  [4096, 1024] fp32  per core (shard of x, transposed)
  wT  [4096, 4096] int32 (weight_int8.T, replicated)
  bias_rep [128, 4096] fp32 (bias broadcast to 128 partitions)
  scale_rep [128, 1] fp32
Output per core: out [1024, 4096] fp32.
"""

import os

import numpy as np

import concourse.bass as bass
import concourse.mybir as mybir
import concourse.tile as tile
from concourse.bass_utils import run_bass_kernel_spmd

# ---------------------------------------------------------------------------
# Workaround for this walrus build: it rejects instructions carrying more
# than one sync wait command ("Too many sync wait commands",
# CoreV*GenImpl::setupSyncWait). After Tile lowering, split any instruction
# with >1 sem wait: keep the last wait on the instruction and hoist the rest
# onto same-engine NoOps inserted immediately before it (engines execute
# their stream in order, so this is semantics-preserving).
# ---------------------------------------------------------------------------


def _split_sync_waits(nc, max_waits=1):
    for fn in nc.m.functions:
        for bb in fn.blocks:
            out = []
            changed = False
            for inst in bb.instructions:
                si = inst.sync_info
                waits = list(si.on_wait) if si is not None and si.on_wait else []
                if len(waits) > max_waits:
                    changed = True
                    for w in waits[:-max_waits]:
                        nop = mybir.InstNoOp(
                            name=nc.get_next_instruction_name(),
                            sync_info=mybir.SyncInfo(on_wait=[w], on_update=[]),
                            bass_nofuse=True,
                            engine=inst.engine,
                        )
                        nc.register_instruction(nop)
                        out.append(nop)
                    inst.sync_info = mybir.SyncInfo(
                        on_wait=waits[-max_waits:],
                        on_update=list(si.on_update or []),
                    )
                out.append(inst)
            if changed:
                bb.instructions = out


# ---------------------------------------------------------------------------

P = 128
N_CORES = 8

OUT_F, IN_F = 4096, 4096
B, S = 4, 2048
M_TOTAL = B * S           # 8192 rows
M = M_TOTAL // N_CORES    # 1024 rows per core

F32 = mybir.dt.float32
F16 = mybir.dt.float16
I8 = mybir.dt.int8

# exposed for test.py
LAST_RESULTS = None


def build_nc(M=M, K=IN_F, N=OUT_F, NF=512, trn_type=None):
    """Build the per-core Bass module."""
    KO = K // P    # k chunks of 128
    MT = M // P    # m tiles of 128
    NT = N // NF   # n tiles of NF

    nc = bass.Bass() if trn_type is None else bass.Bass(trn_type)
    xT = nc.dram_tensor("xT", [K, M], F32, kind="ExternalInput")
    wT = nc.dram_tensor("wT", [K, N], I8, kind="ExternalInput")
    bias_rep = nc.dram_tensor("bias_rep", [P, N], F32, kind="ExternalInput")
    scale_rep = nc.dram_tensor("scale_rep", [P, 1], F32, kind="ExternalInput")
    out = nc.dram_tensor("out", [M, N], F32, kind="ExternalOutput")

    x_view = xT.rearrange("(ko p) m -> p ko m", p=P)     # [P, KO, M]
    w_view = wT.rearrange("(ko p) n -> p ko n", p=P)     # [P, KO, N]
    out_view = out.rearrange("(mt p) n -> p mt n", p=P)  # [P, MT, N]

    with tile.TileContext(nc) as tc:
        with (
            tc.tile_pool(name="const", bufs=1) as const_pool,
            tc.tile_pool(name="x16", bufs=1) as x16_pool,
            tc.tile_pool(name="xstage", bufs=6) as xstage_pool,
            tc.tile_pool(name="w16", bufs=2) as w16_pool,
            tc.tile_pool(name="wstage", bufs=4) as wstage_pool,
            tc.tile_pool(name="outsb", bufs=4) as out_pool,
            tc.tile_pool(name="psum", bufs=8, space="PSUM") as psum_pool,
        ):
            bias_sb = const_pool.tile([P, N], F32)
            nc.sync.dma_start(bias_sb[:], bias_rep[:])
            scale_sb = const_pool.tile([P, 1], F32)
            nc.sync.dma_start(scale_sb[:], scale_rep[:])

            # x: load fp32, cast to fp16 resident tile. Startup is paced by
            # this pipeline: spread the DMAs over both DMA paths (HWDGE via
            # sync, SWDGE via gpsimd) and the casts over Scalar+Vector so
            # chunks arrive faster than the PE consumes them.
            x16 = x16_pool.tile([P, KO, M], F16)
            for ko in range(KO):
                xs = xstage_pool.tile([P, M], F32)
                dma_eng = nc.sync if ko % 2 == 0 else nc.gpsimd
                dma_eng.dma_start(xs[:], x_view[:, ko, :])
                cast_eng = nc.vector if ko % 2 == 0 else nc.scalar
                if cast_eng is nc.vector:
                    nc.vector.tensor_copy(x16[:, ko, :], xs[:])
                else:
                    nc.scalar.copy(x16[:, ko, :], xs[:])

            def evict(ps, mt, nt):
                ot = out_pool.tile([P, NF], F32)
                # ot = psum * scale
                nc.scalar.activation(
                    ot[:], ps[:], mybir.ActivationFunctionType.Copy,
                    scale=scale_sb[:],
                )
                # ot += bias
                nc.vector.tensor_add(
                    ot[:], ot[:], bias_sb[:, nt * NF : (nt + 1) * NF]
                )
                nc.sync.dma_start(
                    out_view[:, mt, nt * NF : (nt + 1) * NF], ot[:]
                )

            for nt in range(NT):
                # weight n-slice: load int8, cast to fp16 (exact for +-127)
                w16 = w16_pool.tile([P, KO, NF], F16)
                if nt == 0:
                    # Startup is DMA-paced: go k-outer / m-inner across all
                    # 8 PSUM banks so each arriving (x,w) k-chunk unlocks
                    # MT matmuls instead of one.
                    pss = [
                        psum_pool.tile([P, NF], F32, tag="ps", name=f"ps0_{mt}")
                        for mt in range(MT)
                    ]
                    for ko in range(KO):
                        ws = wstage_pool.tile([P, NF], I8)
                        nc.sync.dma_start(ws[:], w_view[:, ko, 0:NF])
                        nc.vector.tensor_copy(w16[:, ko, :], ws[:])
                        for mt in range(MT):
                            nc.tensor.matmul(
                                pss[mt][:],
                                x16[:, ko, mt * P : (mt + 1) * P],
                                w16[:, ko, :],
                                start=(ko == 0),
                                stop=(ko == KO - 1),
                            )
                    for mt in range(MT):
                        evict(pss[mt], mt, 0)
                else:
                    for ko in range(KO):
                        ws = wstage_pool.tile([P, NF], I8)
                        nc.sync.dma_start(
                            ws[:], w_view[:, ko, nt * NF : (nt + 1) * NF]
                        )
                        nc.vector.tensor_copy(w16[:, ko, :], ws[:])

                    for mt in range(MT):
                        ps = psum_pool.tile([P, NF], F32, tag="ps", name=f"ps{nt}_{mt}")
                        for ko in range(KO):
                            nc.tensor.matmul(
                                ps[:],
                                x16[:, ko, mt * P : (mt + 1) * P],
                                w16[:, ko, :],
                                start=(ko == 0),
                                stop=(ko == KO - 1),
                            )
                        evict(ps, mt, nt)

    _split_sync_waits(nc)
    return nc


def kernel(x, weight_int8, scale, bias):
    global LAST_RESULTS
    x = np.asarray(x)
    weight_int8 = np.asarray(weight_int8)
    scale = np.asarray(scale)
    bias = np.asarray(bias)

    x2d = np.ascontiguousarray(x.reshape(M_TOTAL, IN_F).astype(np.float32, copy=False))
    # int8 range [-127,127]: the int8 pack is exact (this is the module's
    # decoded-int8 weight; int32 is just the container dtype)
    wT = np.ascontiguousarray(weight_int8.T.astype(np.int8))
    bias_rep = np.ascontiguousarray(
        np.broadcast_to(bias.astype(np.float32, copy=False), (P, OUT_F))
    )
    scale_rep = np.full((P, 1), np.float32(scale), dtype=np.float32)

    in_maps = []
    for c in range(N_CORES):
        shard = x2d[c * M : (c + 1) * M]
        in_maps.append(
            {
                "xT": np.ascontiguousarray(shard.T),
                "wT": wT,
                "bias_rep": bias_rep,
                "scale_rep": scale_rep,
            }
        )

    nc = build_nc()
    trace = bool(int(os.environ.get("KERNEL_TRACE", "0")))
    res = run_bass_kernel_spmd(nc, in_maps, list(range(N_CORES)), trace=trace)
    LAST_RESULTS = res

    out = np.empty((M_TOTAL, OUT_F), dtype=np.float32)
    for c in range(N_CORES):
        out[c * M : (c + 1) * M] = res.results[c]["out"]
    return out.reshape(B, S, OUT_F)


# revision 15
# speedup vs baseline: 1.1159x; 1.0493x over previous
"""Trainium2 kernel for nn_CompressedLinearRANS: out = x @ (w_int8*scale).T + bias.

Strategy: data-parallel over the 8192 rows of x (1024 rows per core, full
weight replicated per core). Per-core matmul M=1024, K=4096, N=4096 done in
fp16 on the PE array: int8-range weights are exactly representable in fp16,
x is rounded fp32->fp16 on device (the original module computes in fp16).
Scale+bias are applied in fp32 during PSUM eviction.

Inputs are pre-transposed on the host so no on-device transposes are needed:
  xT  [4096, 1024] fp32  per core (shard of x, transposed)
  wT  [4096, 4096] int32 (weight_int8.T, replicated)
  bias_rep [128, 4096] fp32 (bias broadcast to 128 partitions)
  scale_rep [128, 1] fp32
Output per core: out [1024, 4096] fp32.
"""

import os

import numpy as np

import concourse.bass as bass
import concourse.mybir as mybir
import concourse.tile as tile
from concourse.bass_utils import run_bass_kernel_spmd

# ---------------------------------------------------------------------------
# Workaround for this walrus build: it rejects instructions carrying more
# than one sync wait command ("Too many sync wait commands",
# CoreV*GenImpl::setupSyncWait). After Tile lowering, split any instruction
# with >1 sem wait: keep the last wait on the instruction and hoist the rest
# onto same-engine NoOps inserted immediately before it (engines execute
# their stream in order, so this is semantics-preserving).
# ---------------------------------------------------------------------------


def _split_sync_waits(nc, max_waits=1):
    for fn in nc.m.functions:
        for bb in fn.blocks:
            out = []
            changed = False
            for inst in bb.instructions:
                si = inst.sync_info
                waits = list(si.on_wait) if si is not None and si.on_wait else []
                if len(waits) > max_waits:
                    changed = True
                    for w in waits[:-max_waits]:
                        nop = mybir.InstNoOp(
                            name=nc.get_next_instruction_name(),
                            sync_info=mybir.SyncInfo(on_wait=[w], on_update=[]),
                            bass_nofuse=True,
                            engine=inst.engine,
                        )
                        nc.register_instruction(nop)
                        out.append(nop)
                    inst.sync_info = mybir.SyncInfo(
                        on_wait=waits[-max_waits:],
                        on_update=list(si.on_update or []),
                    )
                out.append(inst)
            if changed:
                bb.instructions = out


# ---------------------------------------------------------------------------

P = 128
N_CORES = 8

OUT_F, IN_F = 4096, 4096
B, S = 4, 2048
M_TOTAL = B * S           # 8192 rows
M = M_TOTAL // N_CORES    # 1024 rows per core

F32 = mybir.dt.float32
F16 = mybir.dt.float16
I8 = mybir.dt.int8

# exposed for test.py
LAST_RESULTS = None


def build_nc(M=M, K=IN_F, N=OUT_F, NF=512, trn_type=None):
    """Build the per-core Bass module."""
    KO = K // P    # k chunks of 128
    MT = M // P    # m tiles of 128
    NT = N // NF   # n tiles of NF

    nc = bass.Bass() if trn_type is None else bass.Bass(trn_type)
    xT = nc.dram_tensor("xT", [K, M], F32, kind="ExternalInput")
    wT = nc.dram_tensor("wT", [K, N], I8, kind="ExternalInput")
    bias_rep = nc.dram_tensor("bias_rep", [P, N], F32, kind="ExternalInput")
    scale_rep = nc.dram_tensor("scale_rep", [P, 1], F32, kind="ExternalInput")
    out = nc.dram_tensor("out", [M, N], F32, kind="ExternalOutput")

    x_view = xT.rearrange("(ko p) m -> p ko m", p=P)     # [P, KO, M]
    w_view = wT.rearrange("(ko p) n -> p ko n", p=P)     # [P, KO, N]
    out_view = out.rearrange("(mt p) n -> p mt n", p=P)  # [P, MT, N]

    with tile.TileContext(nc) as tc:
        with (
            tc.tile_pool(name="const", bufs=1) as const_pool,
            tc.tile_pool(name="x16", bufs=1) as x16_pool,
            tc.tile_pool(name="xstage", bufs=6) as xstage_pool,
            tc.tile_pool(name="w16", bufs=2) as w16_pool,
            tc.tile_pool(name="wstage", bufs=4) as wstage_pool,
            tc.tile_pool(name="outsb", bufs=4) as out_pool,
            tc.tile_pool(name="psum", bufs=8, space="PSUM") as psum_pool,
        ):
            bias_sb = const_pool.tile([P, N], F32)
            nc.sync.dma_start(bias_sb[:], bias_rep[:])
            scale_sb = const_pool.tile([P, 1], F32)
            nc.sync.dma_start(scale_sb[:], scale_rep[:])

            # Startup critical path: the first n-tile's weights (small, on
            # sync ring + vector casts, emitted FIRST so they lead the
            # vector stream), then the x chunks. x DMAs go on the two DMA
            # paths sync doesn't use (gpsimd SWDGE + scalar's HWDGE ring);
            # x casts split vector(even)/scalar(odd) so each chunk's
            # DMA->cast->matmul pipeline stays ahead of the PE.
            w16_first = w16_pool.tile([P, KO, NF], F16, tag="w16", name="w16_first")
            for ko in range(KO):
                ws = wstage_pool.tile([P, NF], I8, tag="ws", name=f"ws0_{ko}")
                nc.sync.dma_start(ws[:], w_view[:, ko, 0:NF])
                nc.vector.tensor_copy(w16_first[:, ko, :], ws[:])

            x16 = x16_pool.tile([P, KO, M], F16)
            for ko in range(KO):
                xs = xstage_pool.tile([P, M], F32)
                dma_eng = nc.gpsimd if ko % 2 == 0 else nc.scalar
                dma_eng.dma_start(xs[:], x_view[:, ko, :])
                if ko % 2 == 0:
                    nc.vector.tensor_copy(x16[:, ko, :], xs[:])
                else:
                    nc.scalar.copy(x16[:, ko, :], xs[:])

            def evict(ps, mt, nt):
                ot = out_pool.tile([P, NF], F32)
                # ot = psum * scale
                nc.scalar.activation(
                    ot[:], ps[:], mybir.ActivationFunctionType.Copy,
                    scale=scale_sb[:],
                )
                # ot += bias
                nc.vector.tensor_add(
                    ot[:], ot[:], bias_sb[:, nt * NF : (nt + 1) * NF]
                )
                nc.sync.dma_start(
                    out_view[:, mt, nt * NF : (nt + 1) * NF], ot[:]
                )

            # First n-tile: k-outer / m-inner across all 8 PSUM banks so each
            # arriving (x,w) k-chunk unlocks MT matmuls instead of one.
            pss = [
                psum_pool.tile([P, NF], F32, tag="ps", name=f"ps0_{mt}")
                for mt in range(MT)
            ]
            for ko in range(KO):
                for mt in range(MT):
                    nc.tensor.matmul(
                        pss[mt][:],
                        x16[:, ko, mt * P : (mt + 1) * P],
                        w16_first[:, ko, :],
                        start=(ko == 0),
                        stop=(ko == KO - 1),
                    )
            for mt in range(MT):
                evict(pss[mt], mt, 0)

            for nt in range(1, NT):
                # weight n-slice: load int8, cast to fp16 (exact for +-127)
                w16 = w16_pool.tile([P, KO, NF], F16, tag="w16", name=f"w16_{nt}")
                for ko in range(KO):
                    ws = wstage_pool.tile([P, NF], I8, tag="ws", name=f"ws{nt}_{ko}")
                    nc.sync.dma_start(
                        ws[:], w_view[:, ko, nt * NF : (nt + 1) * NF]
                    )
                    nc.vector.tensor_copy(w16[:, ko, :], ws[:])

                for mt in range(MT):
                    ps = psum_pool.tile([P, NF], F32, tag="ps", name=f"ps{nt}_{mt}")
                    for ko in range(KO):
                        nc.tensor.matmul(
                            ps[:],
                            x16[:, ko, mt * P : (mt + 1) * P],
                            w16[:, ko, :],
                            start=(ko == 0),
                            stop=(ko == KO - 1),
                        )
                    evict(ps, mt, nt)

    _split_sync_waits(nc)
    return nc


def kernel(x, weight_int8, scale, bias):
    global LAST_RESULTS
    x = np.asarray(x)
    weight_int8 = np.asarray(weight_int8)
    scale = np.asarray(scale)
    bias = np.asarray(bias)

    x2d = np.ascontiguousarray(x.reshape(M_TOTAL, IN_F).astype(np.float32, copy=False))
    # int8 range [-127,127]: the int8 pack is exact (this is the module's
    # decoded-int8 weight; int32 is just the container dtype)
    wT = np.ascontiguousarray(weight_int8.T.astype(np.int8))
    bias_rep = np.ascontiguousarray(
        np.broadcast_to(bias.astype(np.float32, copy=False), (P, OUT_F))
    )
    scale_rep = np.full((P, 1), np.float32(scale), dtype=np.float32)

    in_maps = []
    for c in range(N_CORES):
        shard = x2d[c * M : (c + 1) * M]
        in_maps.append(
            {
                "xT": np.ascontiguousarray(shard.T),
                "wT": wT,
                "bias_rep": bias_rep,
                "scale_rep": scale_rep,
            }
        )

    nc = build_nc()
    trace = bool(int(os.environ.get("KERNEL_TRACE", "0")))
    res = run_bass_kernel_spmd(nc, in_maps, list(range(N_CORES)), trace=trace)
    LAST_RESULTS = res

    out = np.empty((M_TOTAL, OUT_F), dtype=np.float32)
    for c in range(N_CORES):
        out[c * M : (c + 1) * M] = res.results[c]["out"]
    return out.reshape(B, S, OUT_F)
